# revision 1
# baseline (speedup 1.0000x reference)
"""Bahdanau attention (gumbel-softmax) Trainium2 kernel.

Data-parallel over the batch dim N across 8 NeuronCores (4 batches per core).
Per batch, a single pass over `key` (the only large tensor):

  per 128-row key tile (t-tile):
    PE:  transpose key tile  [t,c] -> [c,t]            (2x 128-col chunks)
    PE:  kp[u,t]   = w_k.T @ keyT                      (contract c, 2 chunks)
    ACT: tanh_ut   = tanh(kp + qp[n] + b)              (fused per-partition bias)
    PE:  score[t,1]= tanh_ut.T @ v                     (contract u)
    ACT: e[t]      = exp(score + gumbel[t])            (no max-subtraction; fp32
                                                        range covers exp(score+gumbel))
    PE:  ctx[1,c] += e.T @ key_tile                    (online softmax numerator)
  batch epilogue: denom = sum(e); ctx/denom, e/denom; DMA out.

Host side does only the tiny query-path projection (q @ w_conv @ w_q + b,
~6 MFLOP on 32x256 data), input sharding/layout, and output gather.
"""

import numpy as np
from contextlib import ExitStack

from concourse import bacc, masks, mybir, tile
from concourse.bass_utils import run_bass_kernel_spmd

F32 = mybir.dt.float32
AF = mybir.ActivationFunctionType
AX = mybir.AxisListType

N, T, C, U = 32, 8192, 256, 128
NCORES = 8
NB = N // NCORES            # batches per core
P = 128                     # partition / tile size
NT = T // P                 # 64 t-tiles per batch
GROUP = 4                   # t-tiles per compute group
NG = NT // GROUP            # 16 groups per batch
BLK = 8                     # t-tiles per DMA block (1 MiB transfers)
NBLK = NT // BLK            # 8 DMA blocks per batch


def _emit_kernel(ctx: ExitStack, tc, key_d, wk_d, v_d, qpb_d, gum_d, ctx_d, aln_d):
    nc = tc.nc

    const = ctx.enter_context(tc.tile_pool(name="const", bufs=1))
    wk_sb = const.tile([P, 2, U], F32)          # [c-in-chunk, c-chunk, u]
    nc.sync.dma_start(wk_sb[:, 0, :], wk_d[0:P, :])
    nc.sync.dma_start(wk_sb[:, 1, :], wk_d[P : 2 * P, :])
    v_sb = const.tile([U, 1], F32)
    nc.sync.dma_start(v_sb[:], v_d[:])
    qpb_sb = const.tile([U, NB], F32)
    nc.sync.dma_start(qpb_sb[:], qpb_d[:])
    ident = const.tile([P, P], F32)
    masks.make_identity(nc, ident[:])
    ones_col = const.tile([P, 1], F32)
    nc.gpsimd.memset(ones_col[:], 1.0)
    ones_row = const.tile([1, P], F32)
    nc.gpsimd.memset(ones_row[:], 1.0)

    gum_pool = ctx.enter_context(tc.tile_pool(name="gum", bufs=2))
    key_pool = ctx.enter_context(tc.tile_pool(name="key", bufs=3))
    keyT_pool = ctx.enter_context(tc.tile_pool(name="keyT", bufs=2))
    tanh_pool = ctx.enter_context(tc.tile_pool(name="tanh", bufs=3))
    e_pool = ctx.enter_context(tc.tile_pool(name="e", bufs=2))
    z_pool = ctx.enter_context(tc.tile_pool(name="z", bufs=2))
    aout_pool = ctx.enter_context(tc.tile_pool(name="aout", bufs=2))
    cout_pool = ctx.enter_context(tc.tile_pool(name="cout", bufs=2))
    mini_pool = ctx.enter_context(tc.tile_pool(name="mini", bufs=2))

    psT_pool = ctx.enter_context(tc.tile_pool(name="psT", bufs=2, space="PSUM"))
    kp_pool = ctx.enter_context(tc.tile_pool(name="kp", bufs=2, space="PSUM"))
    sc_pool = ctx.enter_context(tc.tile_pool(name="sc", bufs=2, space="PSUM"))
    cx_pool = ctx.enter_context(tc.tile_pool(name="cx", bufs=1, space="PSUM"))
    ep_pool = ctx.enter_context(tc.tile_pool(name="ep", bufs=1, space="PSUM"))

    for n in range(NB):
        gum_sb = gum_pool.tile([P, NT], F32)
        nc.sync.dma_start(gum_sb[:], gum_d[n])
        e_sb = e_pool.tile([P, NT], F32)
        cx_ps = cx_pool.tile([1, C], F32)

        key_tiles = {}
        tanh_tiles = {}

        def load_block(b):
            key_sb = key_pool.tile([P, BLK, C], F32)
            src = key_d[n, b * BLK * P : (b + 1) * BLK * P, :].rearrange(
                "(j p) c -> p j c", p=P
            )
            nc.sync.dma_start(key_sb[:], src)
            key_tiles[b] = key_sb

        def front(g):
            # transposes + kp matmul + tanh for group g
            b, h = divmod(g, BLK // GROUP)
            key_sb = key_tiles[b]
            keyT_sb = keyT_pool.tile([P, 2, GROUP * P], F32)
            for pr in range(GROUP // 2):
                psT = psT_pool.tile([P, 2, 2, P], F32)  # [c-chunk, tile-in-pair, col]
                for cc in range(2):
                    for tt in range(2):
                        j = h * GROUP + pr * 2 + tt
                        nc.tensor.matmul(
                            psT[:, cc, tt, :],
                            key_sb[:, j, cc * P : (cc + 1) * P],
                            ident[:],
                            is_transpose=True,
                            start=(cc == 0 and tt == 0),
                            stop=(cc == 1 and tt == 1),
                        )
                dst = keyT_sb[:, :, pr * 2 * P : (pr + 1) * 2 * P].rearrange(
                    "p c (t x) -> p c t x", t=2
                )
                nc.vector.tensor_copy(dst, psT[:, :, :, :])
            kp_ps = kp_pool.tile([U, GROUP * P], F32)
            for cc in range(2):
                nc.tensor.matmul(
                    kp_ps[:],
                    wk_sb[:, cc, :],
                    keyT_sb[:, cc, :],
                    start=(cc == 0),
                    stop=(cc == 1),
                )
            th_sb = tanh_pool.tile([U, GROUP * P], F32)
            nc.scalar.activation(
                th_sb[:], kp_ps[:], AF.Tanh, bias=qpb_sb[:, n : n + 1]
            )
            tanh_tiles[g] = th_sb

        def score(g):
            th_sb = tanh_tiles.pop(g)
            sc_ps = sc_pool.tile([P, GROUP], F32)
            for j in range(GROUP):
                nc.tensor.matmul(
                    sc_ps[:, j : j + 1],
                    th_sb[:, j * P : (j + 1) * P],
                    v_sb[:],
                    start=(j == 0),
                    stop=(j == GROUP - 1),
                )
            z_sb = z_pool.tile([P, GROUP], F32)
            nc.vector.tensor_add(
                z_sb[:], sc_ps[:], gum_sb[:, g * GROUP : (g + 1) * GROUP]
            )
            nc.scalar.activation(
                e_sb[:, g * GROUP : (g + 1) * GROUP], z_sb[:], AF.Exp
            )

        def ctx_acc(g):
            b, h = divmod(g, BLK // GROUP)
            key_sb = key_tiles[b]
            for j in range(GROUP):
                i = g * GROUP + j
                nc.tensor.matmul(
                    cx_ps[:],
                    e_sb[:, i : i + 1],
                    key_sb[:, h * GROUP + j, :],
                    start=(i == 0),
                    stop=(i == NT - 1),
                )

        # software-pipelined emission: PE order per iter g is
        #   T(g) kp(g) | cx(g-2) | sc(g-1)
        # so each cross-engine round-trip has a full group of PE work to hide in.
        load_block(0)
        for g in range(NG + 2):
            if g < NG:
                if g % (BLK // GROUP) == 0:
                    b_next = g // (BLK // GROUP) + 1
                    if b_next < NBLK:
                        load_block(b_next)
                front(g)
            if g >= 2:
                ctx_acc(g - 2)
            if 1 <= g <= NG:
                score(g - 1)

        # batch epilogue: denominator + normalization
        den_ps = ep_pool.tile([1, NT], F32, tag="ep")
        nc.tensor.matmul(den_ps[:], ones_col[:], e_sb[:], start=True, stop=True)
        s_sb = mini_pool.tile([1, 1], F32, tag="ssum")
        nc.vector.reduce_sum(s_sb[:], den_ps[:], axis=AX.X)
        r_sb = mini_pool.tile([1, 1], F32, tag="recip")
        nc.vector.reciprocal(r_sb[:], s_sb[:])
        rb_ps = ep_pool.tile([P, 1], F32, tag="ep")
        nc.tensor.matmul(rb_ps[:], ones_row[:], r_sb[:], start=True, stop=True)
        rb_sb = mini_pool.tile([P, 1], F32, tag="rb")
        nc.scalar.copy(rb_sb[:], rb_ps[:])

        aln_sb = aout_pool.tile([P, NT], F32)
        nc.vector.tensor_scalar_mul(aln_sb[:], e_sb[:], rb_sb[:])
        nc.sync.dma_start(aln_d[n], aln_sb[:])
        cxo_sb = cout_pool.tile([1, C], F32)
        nc.vector.tensor_scalar_mul(cxo_sb[:], cx_ps[:], r_sb[:])
        nc.sync.dma_start(ctx_d[n : n + 1, :], cxo_sb[:])


def build_nc():
    nc = bacc.Bacc("TRN2", target_bir_lowering=False)
    key_d = nc.dram_tensor("key", [NB, T, C], F32, kind="ExternalInput")
    wk_d = nc.dram_tensor("wk", [C, U], F32, kind="ExternalInput")
    v_d = nc.dram_tensor("v", [U, 1], F32, kind="ExternalInput")
    qpb_d = nc.dram_tensor("qpb", [U, NB], F32, kind="ExternalInput")
    gum_d = nc.dram_tensor("gum", [NB, P, NT], F32, kind="ExternalInput")
    ctx_d = nc.dram_tensor("ctx", [NB, C], F32, kind="ExternalOutput")
    aln_d = nc.dram_tensor("aln", [NB, P, NT], F32, kind="ExternalOutput")
    with tile.TileContext(nc) as tc, ExitStack() as ctx:
        _emit_kernel(ctx, tc, key_d, wk_d, v_d, qpb_d, gum_d, ctx_d, aln_d)
    nc.compile()
    return nc


def make_in_maps(query, key, w_conv, w_q, w_k, v, b, gumbel):
    """Host-side sharding + tiny query-path projection + layout transforms."""
    query = np.asarray(query, np.float32)
    key = np.ascontiguousarray(np.asarray(key, np.float32))
    w_conv = np.asarray(w_conv, np.float32)
    w_q = np.asarray(w_q, np.float32)
    w_k = np.ascontiguousarray(np.asarray(w_k, np.float32))
    v = np.asarray(v, np.float32)
    b = np.asarray(b, np.float32)
    gumbel = np.asarray(gumbel, np.float32)

    qp = (query.reshape(N, C) @ w_conv.T) @ w_q + b.reshape(1, U)  # (N, U)
    # gumbel columns: gum[n, p, i] = gumbel[n, i*128 + p]
    gum_t = np.ascontiguousarray(
        gumbel.reshape(N, NT, P).transpose(0, 2, 1)
    )  # (N, 128, 64)
    v_col = np.ascontiguousarray(v.reshape(U, 1))

    in_maps = []
    for i in range(NCORES):
        sl = slice(i * NB, (i + 1) * NB)
        in_maps.append(
            {
                "key": np.ascontiguousarray(key[sl]),
                "wk": w_k,
                "v": v_col,
                "qpb": np.ascontiguousarray(qp[sl].T),  # (128, NB)
                "gum": np.ascontiguousarray(gum_t[sl]),
            }
        )
    return in_maps


def gather_outputs(results):
    context = np.empty((N, 1, C), np.float32)
    align = np.empty((N, 1, T), np.float32)
    for i, r in enumerate(results):
        sl = slice(i * NB, (i + 1) * NB)
        context[sl, 0, :] = r["ctx"]
        align[sl, 0, :] = r["aln"].transpose(0, 2, 1).reshape(NB, T)
    return context, align


_NC_CACHE = None


def kernel(query, key, w_conv, w_q, w_k, v, b, gumbel):
    global _NC_CACHE
    if _NC_CACHE is None:
        _NC_CACHE = build_nc()
    in_maps = make_in_maps(query, key, w_conv, w_q, w_k, v, b, gumbel)
    res = run_bass_kernel_spmd(_NC_CACHE, in_maps, core_ids=list(range(NCORES)))
    return gather_outputs(res.results)


# revision 5
# speedup vs baseline: 1.8300x; 1.8300x over previous
"""Bahdanau attention (gumbel-softmax) Trainium2 kernel.

Data-parallel over the batch dim N across 8 NeuronCores (4 batches per core).
Per batch, a single pass over `key` (the only large tensor):

  per 128-row key tile (t-tile):
    PE:  transpose key tile  [t,c] -> [c,t]            (2x 128-col chunks)
    PE:  kp[u,t]   = w_k.T @ keyT                      (contract c, 2 chunks)
    ACT: tanh_ut   = tanh(kp + qp[n] + b)              (fused per-partition bias)
    PE:  score[t,1]= tanh_ut.T @ v                     (contract u)
    ACT: e[t]      = exp(score + gumbel[t])            (no max-subtraction; fp32
                                                        range covers exp(score+gumbel))
    PE:  ctx[1,c] += e.T @ key_tile                    (online softmax numerator)
  batch epilogue: denom = sum(e); ctx/denom, e/denom; DMA out.

Host side does only the tiny query-path projection (q @ w_conv @ w_q + b,
~6 MFLOP on 32x256 data), input sharding/layout, and output gather.
"""

import numpy as np
from contextlib import ExitStack

from concourse import bacc, masks, mybir, tile
from concourse.bass_utils import run_bass_kernel_spmd

F32 = mybir.dt.float32
F32R = mybir.dt.float32r
AF = mybir.ActivationFunctionType
AX = mybir.AxisListType

N, T, C, U = 32, 8192, 256, 128
NCORES = 8
NB = N // NCORES            # batches per core
P = 128                     # partition / tile size
NT = T // P                 # 64 t-tiles per batch
GROUP = 4                   # t-tiles per compute group
NG = NT // GROUP            # 16 groups per batch
BLK = 8                     # t-tiles per DMA block (1 MiB transfers)
NBLK = NT // BLK            # 8 DMA blocks per batch


def _emit_kernel(ctx: ExitStack, tc, key_d, wk_d, v_d, qpb_d, gum_d, ctx_d, aln_d):
    nc = tc.nc

    const = ctx.enter_context(tc.tile_pool(name="const", bufs=1))
    wk_sb = const.tile([P, 2, U], F32R)          # [c-in-chunk, c-chunk, u]
    nc.sync.dma_start(wk_sb[:, 0, :], wk_d[0:P, :])
    nc.sync.dma_start(wk_sb[:, 1, :], wk_d[P : 2 * P, :])
    v2_sb = const.tile([U, 2], F32R)
    nc.sync.dma_start(v2_sb[:], v_d[:])
    qpb_sb = const.tile([U, NB], F32)
    nc.sync.dma_start(qpb_sb[:], qpb_d[:])
    ident32 = const.tile([P, P], F32)
    masks.make_identity(nc, ident32[:])
    ident = const.tile([P, P], F32R)
    nc.vector.tensor_copy(ident[:], ident32[:])
    ones32 = const.tile([P, 1], F32)
    nc.gpsimd.memset(ones32[:], 1.0)
    ones_col = const.tile([P, 1], F32R)
    nc.vector.tensor_copy(ones_col[:], ones32[:])
    ones_row32 = const.tile([1, P], F32)
    nc.gpsimd.memset(ones_row32[:], 1.0)
    ones_row = const.tile([1, P], F32R)
    nc.vector.tensor_copy(ones_row[:], ones_row32[:])

    gum_pool = ctx.enter_context(tc.tile_pool(name="gum", bufs=2))
    key_pool = ctx.enter_context(tc.tile_pool(name="key", bufs=3))
    keyT_pool = ctx.enter_context(tc.tile_pool(name="keyT", bufs=2))
    tanh_pool = ctx.enter_context(tc.tile_pool(name="tanh", bufs=3))
    e_pool = ctx.enter_context(tc.tile_pool(name="e", bufs=2))
    z_pool = ctx.enter_context(tc.tile_pool(name="z", bufs=2))
    aout_pool = ctx.enter_context(tc.tile_pool(name="aout", bufs=2))
    cout_pool = ctx.enter_context(tc.tile_pool(name="cout", bufs=2))
    mini_pool = ctx.enter_context(tc.tile_pool(name="mini", bufs=2))

    psT_pool = ctx.enter_context(tc.tile_pool(name="psT", bufs=2, space="PSUM"))
    kp_pool = ctx.enter_context(tc.tile_pool(name="kp", bufs=2, space="PSUM"))
    sc_pool = ctx.enter_context(tc.tile_pool(name="sc", bufs=2, space="PSUM"))
    cx_pool = ctx.enter_context(tc.tile_pool(name="cx", bufs=1, space="PSUM"))
    ep_pool = ctx.enter_context(tc.tile_pool(name="ep", bufs=1, space="PSUM"))

    for n in range(NB):
        gum_sb = gum_pool.tile([P, NT], F32)
        nc.sync.dma_start(gum_sb[:], gum_d[n])
        e_sb = e_pool.tile([P, NT], F32R)
        cx_ps = cx_pool.tile([1, C], F32)

        key_tiles = {}
        tanh_tiles = {}

        def load_block(b):
            key_sb = key_pool.tile([P, BLK, C], F32R)
            src = key_d[n, b * BLK * P : (b + 1) * BLK * P, :].rearrange(
                "(j p) c -> p j c", p=P
            )
            nc.sync.dma_start(key_sb[:], src)
            key_tiles[b] = key_sb

        def front(g):
            # transposes + kp matmul + tanh for group g
            b, h = divmod(g, BLK // GROUP)
            key_sb = key_tiles[b]
            keyT_sb = keyT_pool.tile([P, 2, GROUP * P], F32R)
            for pr in range(GROUP // 2):
                psT = psT_pool.tile([P, 2, 2, P], F32R)  # [c-chunk, tile-in-pair, col]
                for cc in range(2):
                    for tt in range(2):
                        j = h * GROUP + pr * 2 + tt
                        nc.tensor.matmul(
                            psT[:, cc, tt, :],
                            key_sb[:, j, cc * P : (cc + 1) * P],
                            ident[:],
                            is_transpose=True,
                            start=(cc == 0 and tt == 0),
                            stop=(cc == 1 and tt == 1),
                        )
                dst = keyT_sb[:, :, pr * 2 * P : (pr + 1) * 2 * P].rearrange(
                    "p c (t x) -> p c t x", t=2
                )
                nc.vector.tensor_copy(dst, psT[:, :, :, :])
            kp_ps = kp_pool.tile([U, GROUP * P], F32)
            for cc in range(2):
                nc.tensor.matmul(
                    kp_ps[:],
                    wk_sb[:, cc, :],
                    keyT_sb[:, cc, :],
                    start=(cc == 0),
                    stop=(cc == 1),
                )
            th_sb = tanh_pool.tile([U, GROUP * P], F32R)
            nc.scalar.activation(
                th_sb[:], kp_ps[:], AF.Tanh, bias=qpb_sb[:, n : n + 1]
            )
            tanh_tiles[g] = th_sb

        def score(g):
            th_sb = tanh_tiles.pop(g)
            sc_ps = sc_pool.tile([P, GROUP, 2], F32)
            for j in range(GROUP):
                nc.tensor.matmul(
                    sc_ps[:, j, :],
                    th_sb[:, j * P : (j + 1) * P],
                    v2_sb[:],
                    start=(j == 0),
                    stop=(j == GROUP - 1),
                )
            z_sb = z_pool.tile([P, GROUP], F32)
            nc.vector.tensor_add(
                z_sb[:], sc_ps[:, :, 0], gum_sb[:, g * GROUP : (g + 1) * GROUP]
            )
            nc.scalar.activation(
                e_sb[:, g * GROUP : (g + 1) * GROUP], z_sb[:], AF.Exp
            )

        def ctx_acc(g):
            b, h = divmod(g, BLK // GROUP)
            key_sb = key_tiles[b]
            for j in range(GROUP):
                i = g * GROUP + j
                nc.tensor.matmul(
                    cx_ps[:],
                    e_sb[:, i : i + 1],
                    key_sb[:, h * GROUP + j, :],
                    start=(i == 0),
                    stop=(i == NT - 1),
                )

        # software-pipelined emission: PE order per iter g is
        #   T(g) kp(g) | cx(g-2) | sc(g-1)
        # so each cross-engine round-trip has a full group of PE work to hide in.
        load_block(0)
        for g in range(NG + 2):
            if g < NG:
                if g % (BLK // GROUP) == 0:
                    b_next = g // (BLK // GROUP) + 1
                    if b_next < NBLK:
                        load_block(b_next)
                front(g)
            if g >= 2:
                ctx_acc(g - 2)
            if 1 <= g <= NG:
                score(g - 1)

        # batch epilogue: denominator + normalization
        den_ps = ep_pool.tile([1, NT], F32, tag="ep")
        nc.tensor.matmul(den_ps[:], ones_col[:], e_sb[:], start=True, stop=True)
        s_sb = mini_pool.tile([1, 1], F32, tag="ssum")
        nc.vector.reduce_sum(s_sb[:], den_ps[:], axis=AX.X)
        r32_sb = mini_pool.tile([1, 1], F32, tag="recip32")
        nc.vector.reciprocal(r32_sb[:], s_sb[:])
        r_sb = mini_pool.tile([1, 2], F32R, tag="recip")
        nc.vector.tensor_copy(r_sb[:, 0:1], r32_sb[:])
        nc.vector.tensor_copy(r_sb[:, 1:2], r32_sb[:])
        rb_ps = ep_pool.tile([P, 2], F32, tag="ep")
        nc.tensor.matmul(rb_ps[:], ones_row[:], r_sb[:], start=True, stop=True)
        rb_sb = mini_pool.tile([P, 1], F32, tag="rb")
        nc.scalar.copy(rb_sb[:], rb_ps[:, 0:1])

        aln_sb = aout_pool.tile([P, NT], F32)
        nc.vector.tensor_scalar_mul(aln_sb[:], e_sb[:].bitcast(F32), rb_sb[:])
        nc.sync.dma_start(aln_d[n], aln_sb[:])
        cxo_sb = cout_pool.tile([1, C], F32)
        nc.vector.tensor_scalar_mul(cxo_sb[:], cx_ps[:], r32_sb[:])
        nc.sync.dma_start(ctx_d[n : n + 1, :], cxo_sb[:])


def build_nc():
    nc = bacc.Bacc("TRN2", target_bir_lowering=False)
    key_d = nc.dram_tensor("key", [NB, T, C], F32R, kind="ExternalInput")
    wk_d = nc.dram_tensor("wk", [C, U], F32R, kind="ExternalInput")
    v_d = nc.dram_tensor("v", [U, 2], F32R, kind="ExternalInput")
    qpb_d = nc.dram_tensor("qpb", [U, NB], F32, kind="ExternalInput")
    gum_d = nc.dram_tensor("gum", [NB, P, NT], F32, kind="ExternalInput")
    ctx_d = nc.dram_tensor("ctx", [NB, C], F32, kind="ExternalOutput")
    aln_d = nc.dram_tensor("aln", [NB, P, NT], F32, kind="ExternalOutput")
    with tile.TileContext(nc) as tc, ExitStack() as ctx:
        _emit_kernel(ctx, tc, key_d, wk_d, v_d, qpb_d, gum_d, ctx_d, aln_d)
    nc.compile()
    return nc


def make_in_maps(query, key, w_conv, w_q, w_k, v, b, gumbel):
    """Host-side sharding + tiny query-path projection + layout transforms."""
    query = np.asarray(query, np.float32)
    key = np.ascontiguousarray(np.asarray(key, np.float32))
    w_conv = np.asarray(w_conv, np.float32)
    w_q = np.asarray(w_q, np.float32)
    w_k = np.ascontiguousarray(np.asarray(w_k, np.float32))
    v = np.asarray(v, np.float32)
    b = np.asarray(b, np.float32)
    gumbel = np.asarray(gumbel, np.float32)

    qp = (query.reshape(N, C) @ w_conv.T) @ w_q + b.reshape(1, U)  # (N, U)
    # gumbel columns: gum[n, p, i] = gumbel[n, i*128 + p]
    gum_t = np.ascontiguousarray(
        gumbel.reshape(N, NT, P).transpose(0, 2, 1)
    )  # (N, 128, 64)
    v_col = np.ascontiguousarray(np.repeat(v.reshape(U, 1), 2, axis=1))

    in_maps = []
    for i in range(NCORES):
        sl = slice(i * NB, (i + 1) * NB)
        in_maps.append(
            {
                "key": np.ascontiguousarray(key[sl]),
                "wk": w_k,
                "v": v_col,
                "qpb": np.ascontiguousarray(qp[sl].T),  # (128, NB)
                "gum": np.ascontiguousarray(gum_t[sl]),
            }
        )
    return in_maps


def gather_outputs(results):
    context = np.empty((N, 1, C), np.float32)
    align = np.empty((N, 1, T), np.float32)
    for i, r in enumerate(results):
        sl = slice(i * NB, (i + 1) * NB)
        context[sl, 0, :] = r["ctx"]
        align[sl, 0, :] = r["aln"].transpose(0, 2, 1).reshape(NB, T)
    return context, align


_NC_CACHE = None


def kernel(query, key, w_conv, w_q, w_k, v, b, gumbel):
    global _NC_CACHE
    if _NC_CACHE is None:
        _NC_CACHE = build_nc()
    in_maps = make_in_maps(query, key, w_conv, w_q, w_k, v, b, gumbel)
    res = run_bass_kernel_spmd(_NC_CACHE, in_maps, core_ids=list(range(NCORES)))
    return gather_outputs(res.results)


# revision 6
# speedup vs baseline: 2.4650x; 1.3470x over previous
"""Bahdanau attention (gumbel-softmax) Trainium2 kernel.

Data-parallel over the batch dim N across 8 NeuronCores (4 batches per core).
Per batch, a single pass over `key` (the only large tensor), fp16 on-chip
compute with fp32 PSUM accumulation:

  per 128-row key tile (t-tile):
    PE:  transpose key tile  [t,c] -> [c,t]            (fp16, 2x 128-col chunks)
    PE:  kp[u,t]   = w_k.T @ keyT                      (contract c, fp32 accum)
    ACT: tanh_ut   = tanh(kp + qp[n] + b)              (fused per-partition bias,
                                                        fp16 out)
    PE:  score[t,2]= tanh_ut.T @ [v v]                 (contract u, fp32 out)
    ACT: e[t]      = exp(score + gumbel[t] - 10)       (shift keeps e in fp16
                                                        range; cancels in softmax)
    PE:  ctx[1,c] += e16.T @ key_tile                  (online softmax numerator)
  batch epilogue: denom = sum(e); ctx/denom, e/denom; DMA out.

key is cast to fp16 on host (halves HBM traffic; target_regime=memory) and
rows are assigned to partitions p-major (t = blk*1024 + p*8 + j) so each DMA
partition burst is 8KB contiguous. The row permutation cancels: score and
context contract over the same permuted rows, and host-side layout transforms
of gumbel / align account for it. All reductions accumulate in fp32; align is
produced from the fp32 exp. The gumbel shift (-10) cancels exactly in the
softmax normalization.

Host side does only the tiny query-path projection (q @ w_conv @ w_q + b,
~6 MFLOP on 32x256 data), dtype/layout preparation, and output gather.
"""

import numpy as np
from contextlib import ExitStack

from concourse import bacc, masks, mybir, tile
from concourse.bass_utils import run_bass_kernel_spmd

F32 = mybir.dt.float32
F16 = mybir.dt.float16
AF = mybir.ActivationFunctionType
AX = mybir.AxisListType

N, T, C, U = 32, 8192, 256, 128
NCORES = 8
NB = N // NCORES            # batches per core
P = 128                     # partition / tile size
NT = T // P                 # 64 t-tiles per batch
GROUP = 4                   # t-tiles per compute group
NG = NT // GROUP            # 16 groups per batch
BLK = 8                     # t-tiles per DMA block (0.5 MiB fp16)
NBLK = NT // BLK            # 8 DMA blocks per batch

GSHIFT = 10.0               # gumbel shift (applied on host) for fp16 exp range


def _emit_kernel(ctx: ExitStack, tc, key_d, wk_d, v_d, qpb_d, gum_d, ctx_d, aln_d):
    nc = tc.nc

    const = ctx.enter_context(tc.tile_pool(name="const", bufs=1))
    wk_sb = const.tile([P, 2, U], F16)          # [c-in-chunk, c-chunk, u]
    nc.sync.dma_start(wk_sb[:, 0, :], wk_d[0:P, :])
    nc.sync.dma_start(wk_sb[:, 1, :], wk_d[P : 2 * P, :])
    v2_sb = const.tile([U, 2], F16)
    nc.sync.dma_start(v2_sb[:], v_d[:])
    qpb_sb = const.tile([U, NB], F32)
    nc.sync.dma_start(qpb_sb[:], qpb_d[:])
    ident = const.tile([P, P], F16)
    masks.make_identity(nc, ident[:])
    ones_col = const.tile([P, 1], F32)
    nc.gpsimd.memset(ones_col[:], 1.0)
    ones_row = const.tile([1, P], F32)
    nc.gpsimd.memset(ones_row[:], 1.0)

    gum_pool = ctx.enter_context(tc.tile_pool(name="gum", bufs=2))
    key_pool = ctx.enter_context(tc.tile_pool(name="key", bufs=3))
    keyT_pool = ctx.enter_context(tc.tile_pool(name="keyT", bufs=2))
    tanh_pool = ctx.enter_context(tc.tile_pool(name="tanh", bufs=3))
    e_pool = ctx.enter_context(tc.tile_pool(name="e", bufs=2))
    z_pool = ctx.enter_context(tc.tile_pool(name="z", bufs=2))
    aout_pool = ctx.enter_context(tc.tile_pool(name="aout", bufs=2))
    cout_pool = ctx.enter_context(tc.tile_pool(name="cout", bufs=2))
    mini_pool = ctx.enter_context(tc.tile_pool(name="mini", bufs=2))

    psT_pool = ctx.enter_context(tc.tile_pool(name="psT", bufs=2, space="PSUM"))
    kp_pool = ctx.enter_context(tc.tile_pool(name="kp", bufs=2, space="PSUM"))
    sc_pool = ctx.enter_context(tc.tile_pool(name="sc", bufs=2, space="PSUM"))
    cx_pool = ctx.enter_context(tc.tile_pool(name="cx", bufs=1, space="PSUM"))
    ep_pool = ctx.enter_context(tc.tile_pool(name="ep", bufs=1, space="PSUM"))

    for n in range(NB):
        gum_sb = gum_pool.tile([P, NT], F32)
        nc.sync.dma_start(gum_sb[:], gum_d[n])
        e_sb = e_pool.tile([P, NT], F32, tag="e32")
        e16_sb = e_pool.tile([P, NT], F16, tag="e16")
        cx_ps = cx_pool.tile([1, C], F32)

        key_tiles = {}
        tanh_tiles = {}

        def load_block(b):
            key_sb = key_pool.tile([P, BLK, C], F16)
            # p-major rows: partition p holds rows b*1024 + p*8 + j (8KB burst)
            src = key_d[n, b * BLK * P : (b + 1) * BLK * P, :].rearrange(
                "(p j) c -> p j c", p=P
            )
            nc.sync.dma_start(key_sb[:], src)
            key_tiles[b] = key_sb

        def front(g):
            # transposes + kp matmul + tanh for group g
            b, h = divmod(g, BLK // GROUP)
            key_sb = key_tiles[b]
            keyT_sb = keyT_pool.tile([P, 2, GROUP * P], F16)
            for pr in range(GROUP // 2):
                psT = psT_pool.tile([P, 2, 2, P], F16)  # [c-chunk, tile-in-pair, col]
                for cc in range(2):
                    for tt in range(2):
                        j = h * GROUP + pr * 2 + tt
                        nc.tensor.matmul(
                            psT[:, cc, tt, :],
                            key_sb[:, j, cc * P : (cc + 1) * P],
                            ident[:],
                            is_transpose=True,
                            start=(cc == 0 and tt == 0),
                            stop=(cc == 1 and tt == 1),
                        )
                dst = keyT_sb[:, :, pr * 2 * P : (pr + 1) * 2 * P].rearrange(
                    "p c (t x) -> p c t x", t=2
                )
                nc.vector.tensor_copy(dst, psT[:, :, :, :])
            kp_ps = kp_pool.tile([U, GROUP * P], F32)
            for cc in range(2):
                nc.tensor.matmul(
                    kp_ps[:],
                    wk_sb[:, cc, :],
                    keyT_sb[:, cc, :],
                    start=(cc == 0),
                    stop=(cc == 1),
                )
            th_sb = tanh_pool.tile([U, GROUP * P], F16)
            nc.scalar.activation(
                th_sb[:], kp_ps[:], AF.Tanh, bias=qpb_sb[:, n : n + 1]
            )
            tanh_tiles[g] = th_sb

        def score(g):
            th_sb = tanh_tiles.pop(g)
            sc_ps = sc_pool.tile([P, GROUP, 2], F32)
            for j in range(GROUP):
                nc.tensor.matmul(
                    sc_ps[:, j, :],
                    th_sb[:, j * P : (j + 1) * P],
                    v2_sb[:],
                    start=(j == 0),
                    stop=(j == GROUP - 1),
                )
            z_sb = z_pool.tile([P, GROUP], F32)
            nc.vector.tensor_add(
                z_sb[:], sc_ps[:, :, 0], gum_sb[:, g * GROUP : (g + 1) * GROUP]
            )
            nc.scalar.activation(
                e_sb[:, g * GROUP : (g + 1) * GROUP], z_sb[:], AF.Exp
            )
            nc.vector.tensor_copy(
                e16_sb[:, g * GROUP : (g + 1) * GROUP],
                e_sb[:, g * GROUP : (g + 1) * GROUP],
            )

        def ctx_acc(g):
            b, h = divmod(g, BLK // GROUP)
            key_sb = key_tiles[b]
            for j in range(GROUP):
                i = g * GROUP + j
                nc.tensor.matmul(
                    cx_ps[:],
                    e16_sb[:, i : i + 1],
                    key_sb[:, h * GROUP + j, :],
                    start=(i == 0),
                    stop=(i == NT - 1),
                )

        # software-pipelined emission: PE order per iter g is
        #   T(g) kp(g) | cx(g-2) | sc(g-1)
        # so each cross-engine round-trip has a full group of PE work to hide in.
        load_block(0)
        for g in range(NG + 2):
            if g < NG:
                if g % (BLK // GROUP) == 0:
                    b_next = g // (BLK // GROUP) + 1
                    if b_next < NBLK:
                        load_block(b_next)
                front(g)
            if g >= 2:
                ctx_acc(g - 2)
            if 1 <= g <= NG:
                score(g - 1)

        # batch epilogue: denominator + normalization (tiny fp32 matmuls)
        den_ps = ep_pool.tile([1, NT], F32, tag="ep")
        nc.tensor.matmul(den_ps[:], ones_col[:], e_sb[:], start=True, stop=True)
        s_sb = mini_pool.tile([1, 1], F32, tag="ssum")
        nc.vector.reduce_sum(s_sb[:], den_ps[:], axis=AX.X)
        r32_sb = mini_pool.tile([1, 1], F32, tag="recip32")
        nc.vector.reciprocal(r32_sb[:], s_sb[:])
        rb_ps = ep_pool.tile([P, 1], F32, tag="ep")
        nc.tensor.matmul(rb_ps[:], ones_row[:], r32_sb[:], start=True, stop=True)
        rb_sb = mini_pool.tile([P, 1], F32, tag="rb")
        nc.scalar.copy(rb_sb[:], rb_ps[:])

        aln_sb = aout_pool.tile([P, NT], F32)
        nc.vector.tensor_scalar_mul(aln_sb[:], e_sb[:], rb_sb[:])
        nc.sync.dma_start(aln_d[n], aln_sb[:])
        cxo_sb = cout_pool.tile([1, C], F32)
        nc.vector.tensor_scalar_mul(cxo_sb[:], cx_ps[:], r32_sb[:])
        nc.sync.dma_start(ctx_d[n : n + 1, :], cxo_sb[:])


def build_nc():
    nc = bacc.Bacc("TRN2", target_bir_lowering=False)
    key_d = nc.dram_tensor("key", [NB, T, C], F16, kind="ExternalInput")
    wk_d = nc.dram_tensor("wk", [C, U], F16, kind="ExternalInput")
    v_d = nc.dram_tensor("v", [U, 2], F16, kind="ExternalInput")
    qpb_d = nc.dram_tensor("qpb", [U, NB], F32, kind="ExternalInput")
    gum_d = nc.dram_tensor("gum", [NB, P, NT], F32, kind="ExternalInput")
    ctx_d = nc.dram_tensor("ctx", [NB, C], F32, kind="ExternalOutput")
    aln_d = nc.dram_tensor("aln", [NB, P, NT], F32, kind="ExternalOutput")
    with tile.TileContext(nc) as tc, ExitStack() as ctx:
        _emit_kernel(ctx, tc, key_d, wk_d, v_d, qpb_d, gum_d, ctx_d, aln_d)
    nc.compile()
    return nc


def make_in_maps(query, key, w_conv, w_q, w_k, v, b, gumbel):
    """Host-side sharding + tiny query-path projection + layout transforms."""
    query = np.asarray(query, np.float32)
    key = np.asarray(key, np.float32)
    w_conv = np.asarray(w_conv, np.float32)
    w_q = np.asarray(w_q, np.float32)
    w_k = np.asarray(w_k, np.float32)
    v = np.asarray(v, np.float32)
    b = np.asarray(b, np.float32)
    gumbel = np.asarray(gumbel, np.float32)

    qp = (query.reshape(N, C) @ w_conv.T) @ w_q + b.reshape(1, U)  # (N, U)
    # gumbel layout matching p-major key rows: t = blk*1024 + p*8 + j,
    # e/gum column index i = blk*8 + j  ->  gum[n, p, blk*8+j] = gumbel[n, t]
    gum_t = np.ascontiguousarray(
        (gumbel.reshape(N, NBLK, P, BLK) - GSHIFT)
        .transpose(0, 2, 1, 3)
        .reshape(N, P, NT)
    )
    key16 = np.ascontiguousarray(key.astype(np.float16))
    wk16 = np.ascontiguousarray(w_k.astype(np.float16))
    v16 = np.ascontiguousarray(
        np.repeat(v.reshape(U, 1), 2, axis=1).astype(np.float16)
    )

    in_maps = []
    for i in range(NCORES):
        sl = slice(i * NB, (i + 1) * NB)
        in_maps.append(
            {
                "key": key16[sl],
                "wk": wk16,
                "v": v16,
                "qpb": np.ascontiguousarray(qp[sl].T),  # (128, NB)
                "gum": np.ascontiguousarray(gum_t[sl]),
            }
        )
    return in_maps


def gather_outputs(results):
    context = np.empty((N, 1, C), np.float32)
    align = np.empty((N, 1, T), np.float32)
    for i, r in enumerate(results):
        sl = slice(i * NB, (i + 1) * NB)
        context[sl, 0, :] = r["ctx"]
        # aln[n, p, blk*8+j] = align[n, blk*1024 + p*8 + j]
        a = r["aln"].reshape(NB, P, NBLK, BLK).transpose(0, 2, 1, 3)
        align[sl, 0, :] = a.reshape(NB, T)
    return context, align


_NC_CACHE = None


def kernel(query, key, w_conv, w_q, w_k, v, b, gumbel):
    global _NC_CACHE
    if _NC_CACHE is None:
        _NC_CACHE = build_nc()
    in_maps = make_in_maps(query, key, w_conv, w_q, w_k, v, b, gumbel)
    res = run_bass_kernel_spmd(_NC_CACHE, in_maps, core_ids=list(range(NCORES)))
    return gather_outputs(res.results)


# revision 7
# speedup vs baseline: 2.5034x; 1.0156x over previous
"""Bahdanau attention (gumbel-softmax) Trainium2 kernel.

Data-parallel over the batch dim N across 8 NeuronCores (4 batches per core).
Per batch, a single pass over `key` (the only large tensor), fp16 on-chip
compute with fp32 PSUM accumulation:

  per 128-row key tile (t-tile):
    PE:  kp[u,t]   = w_k.T @ keyT                      (contract c, fp32 accum)
    ACT: tanh_ut   = tanh(kp + qp[n] + b)              (fused per-partition bias,
                                                        fp16 out)
    PE:  score[t,2]= tanh_ut.T @ [v v]                 (contract u, fp32 out)
    ACT: e[t]      = exp(score + gumbel[t] - 10)       (shift keeps e in fp16
                                                        range; cancels in softmax)
    PE:  ctx[1,c] += e16.T @ key_tile                  (online softmax numerator)
  batch epilogue: denom = sum(e); ctx/denom, e/denom; DMA out.

The kp matmul contracts over channels (needs key as [c, t]) while the context
matmul contracts over time (needs key as [t, c]), so the host ships key in
both layouts, cast to fp16 — 2 x 16.8 MiB per core, the same byte volume as
the single fp32 copy, halving effective HBM pressure per layout
(target_regime=memory). All reductions accumulate in fp32; align is produced
from the fp32 exp. The gumbel shift (-10) cancels exactly in the softmax
normalization.

Host side does only the tiny query-path projection (q @ w_conv @ w_q + b,
~6 MFLOP on 32x256 data), dtype/layout preparation, and output gather.
"""

import numpy as np
from contextlib import ExitStack

from concourse import bacc, mybir, tile
from concourse.bass_utils import run_bass_kernel_spmd

F32 = mybir.dt.float32
F16 = mybir.dt.float16
AF = mybir.ActivationFunctionType
AX = mybir.AxisListType

N, T, C, U = 32, 8192, 256, 128
NCORES = 8
NB = N // NCORES            # batches per core
P = 128                     # partition / tile size
NT = T // P                 # 64 t-tiles per batch
GROUP = 4                   # t-tiles per compute group
NG = NT // GROUP            # 16 groups per batch
BLK = 8                     # t-tiles per DMA block (0.5 MiB fp16 per stream)
NBLK = NT // BLK            # 8 DMA blocks per batch

GSHIFT = 10.0               # gumbel shift (applied on host) for fp16 exp range


def _emit_kernel(ctx: ExitStack, tc, key_d, keyt_d, wk_d, v_d, qpb_d, gum_d,
                 ctx_d, aln_d):
    nc = tc.nc

    const = ctx.enter_context(tc.tile_pool(name="const", bufs=1))
    wk_sb = const.tile([P, 2, U], F16)          # [c-in-chunk, c-chunk, u]
    nc.sync.dma_start(wk_sb[:, 0, :], wk_d[0:P, :])
    nc.sync.dma_start(wk_sb[:, 1, :], wk_d[P : 2 * P, :])
    v2_sb = const.tile([U, 2], F16)
    nc.sync.dma_start(v2_sb[:], v_d[:])
    qpb_sb = const.tile([U, NB], F32)
    nc.sync.dma_start(qpb_sb[:], qpb_d[:])
    ones_col = const.tile([P, 1], F32)
    nc.gpsimd.memset(ones_col[:], 1.0)
    ones_row = const.tile([1, P], F32)
    nc.gpsimd.memset(ones_row[:], 1.0)

    gum_pool = ctx.enter_context(tc.tile_pool(name="gum", bufs=2))
    key_pool = ctx.enter_context(tc.tile_pool(name="key", bufs=3))
    keyt_pool = ctx.enter_context(tc.tile_pool(name="keyt", bufs=3))
    tanh_pool = ctx.enter_context(tc.tile_pool(name="tanh", bufs=3))
    e_pool = ctx.enter_context(tc.tile_pool(name="e", bufs=2))
    z_pool = ctx.enter_context(tc.tile_pool(name="z", bufs=2))
    aout_pool = ctx.enter_context(tc.tile_pool(name="aout", bufs=2))
    cout_pool = ctx.enter_context(tc.tile_pool(name="cout", bufs=2))
    mini_pool = ctx.enter_context(tc.tile_pool(name="mini", bufs=2))

    kp_pool = ctx.enter_context(tc.tile_pool(name="kp", bufs=3, space="PSUM"))
    sc_pool = ctx.enter_context(tc.tile_pool(name="sc", bufs=2, space="PSUM"))
    cx_pool = ctx.enter_context(tc.tile_pool(name="cx", bufs=1, space="PSUM"))
    ep_pool = ctx.enter_context(tc.tile_pool(name="ep", bufs=1, space="PSUM"))

    for n in range(NB):
        gum_sb = gum_pool.tile([P, NT], F32)
        nc.sync.dma_start(gum_sb[:], gum_d[n])
        e_sb = e_pool.tile([P, NT], F32, tag="e32")
        e16_sb = e_pool.tile([P, NT], F16, tag="e16")
        cx_ps = cx_pool.tile([1, C], F32)

        key_tiles = {}
        keyt_tiles = {}
        tanh_tiles = {}

        def load_block(b):
            key_sb = key_pool.tile([P, BLK, C], F16)
            src = key_d[n, b * BLK * P : (b + 1) * BLK * P, :].rearrange(
                "(j p) c -> p j c", p=P
            )
            nc.sync.dma_start(key_sb[:], src)
            key_tiles[b] = key_sb
            keyt_sb = keyt_pool.tile([P, 2, BLK * P], F16)
            srct = keyt_d[n].rearrange("(cc p) t -> p cc t", p=P)[
                :, :, b * BLK * P : (b + 1) * BLK * P
            ]
            nc.sync.dma_start(keyt_sb[:], srct)
            keyt_tiles[b] = keyt_sb

        def front(g):
            # kp matmul + tanh for group g
            b, h = divmod(g, BLK // GROUP)
            keyt_sb = keyt_tiles[b]
            kp_ps = kp_pool.tile([U, GROUP * P], F32)
            for cc in range(2):
                nc.tensor.matmul(
                    kp_ps[:],
                    wk_sb[:, cc, :],
                    keyt_sb[:, cc, h * GROUP * P : (h + 1) * GROUP * P],
                    start=(cc == 0),
                    stop=(cc == 1),
                )
            th_sb = tanh_pool.tile([U, GROUP * P], F16)
            nc.scalar.activation(
                th_sb[:], kp_ps[:], AF.Tanh, bias=qpb_sb[:, n : n + 1]
            )
            tanh_tiles[g] = th_sb

        def score(g):
            th_sb = tanh_tiles.pop(g)
            sc_ps = sc_pool.tile([P, GROUP, 2], F32)
            for j in range(GROUP):
                nc.tensor.matmul(
                    sc_ps[:, j, :],
                    th_sb[:, j * P : (j + 1) * P],
                    v2_sb[:],
                    start=(j == 0),
                    stop=(j == GROUP - 1),
                )
            z_sb = z_pool.tile([P, GROUP], F32)
            nc.vector.tensor_add(
                z_sb[:], sc_ps[:, :, 0], gum_sb[:, g * GROUP : (g + 1) * GROUP]
            )
            nc.scalar.activation(
                e_sb[:, g * GROUP : (g + 1) * GROUP], z_sb[:], AF.Exp
            )
            nc.vector.tensor_copy(
                e16_sb[:, g * GROUP : (g + 1) * GROUP],
                e_sb[:, g * GROUP : (g + 1) * GROUP],
            )

        def ctx_acc(g):
            b, h = divmod(g, BLK // GROUP)
            key_sb = key_tiles[b]
            for j in range(GROUP):
                i = g * GROUP + j
                nc.tensor.matmul(
                    cx_ps[:],
                    e16_sb[:, i : i + 1],
                    key_sb[:, h * GROUP + j, :],
                    start=(i == 0),
                    stop=(i == NT - 1),
                )

        # software-pipelined emission: PE order per iter g is
        #   kp(g) | cx(g-2) | sc(g-1)
        # so each cross-engine round-trip has a group of PE work to hide in.
        load_block(0)
        for g in range(NG + 2):
            if g < NG:
                if g % (BLK // GROUP) == 0:
                    b_next = g // (BLK // GROUP) + 1
                    if b_next < NBLK:
                        load_block(b_next)
                front(g)
            if g >= 2:
                ctx_acc(g - 2)
            if 1 <= g <= NG:
                score(g - 1)

        # batch epilogue: denominator + normalization (tiny fp32 matmuls)
        den_ps = ep_pool.tile([1, NT], F32, tag="ep")
        nc.tensor.matmul(den_ps[:], ones_col[:], e_sb[:], start=True, stop=True)
        s_sb = mini_pool.tile([1, 1], F32, tag="ssum")
        nc.vector.reduce_sum(s_sb[:], den_ps[:], axis=AX.X)
        r32_sb = mini_pool.tile([1, 1], F32, tag="recip32")
        nc.vector.reciprocal(r32_sb[:], s_sb[:])
        rb_ps = ep_pool.tile([P, 1], F32, tag="ep")
        nc.tensor.matmul(rb_ps[:], ones_row[:], r32_sb[:], start=True, stop=True)
        rb_sb = mini_pool.tile([P, 1], F32, tag="rb")
        nc.scalar.copy(rb_sb[:], rb_ps[:])

        aln_sb = aout_pool.tile([P, NT], F32)
        nc.vector.tensor_scalar_mul(aln_sb[:], e_sb[:], rb_sb[:])
        nc.sync.dma_start(aln_d[n], aln_sb[:])
        cxo_sb = cout_pool.tile([1, C], F32)
        nc.vector.tensor_scalar_mul(cxo_sb[:], cx_ps[:], r32_sb[:])
        nc.sync.dma_start(ctx_d[n : n + 1, :], cxo_sb[:])


def build_nc():
    nc = bacc.Bacc("TRN2", target_bir_lowering=False)
    key_d = nc.dram_tensor("key", [NB, T, C], F16, kind="ExternalInput")
    keyt_d = nc.dram_tensor("keyt", [NB, C, T], F16, kind="ExternalInput")
    wk_d = nc.dram_tensor("wk", [C, U], F16, kind="ExternalInput")
    v_d = nc.dram_tensor("v", [U, 2], F16, kind="ExternalInput")
    qpb_d = nc.dram_tensor("qpb", [U, NB], F32, kind="ExternalInput")
    gum_d = nc.dram_tensor("gum", [NB, P, NT], F32, kind="ExternalInput")
    ctx_d = nc.dram_tensor("ctx", [NB, C], F32, kind="ExternalOutput")
    aln_d = nc.dram_tensor("aln", [NB, P, NT], F32, kind="ExternalOutput")
    with tile.TileContext(nc) as tc, ExitStack() as ctx:
        _emit_kernel(ctx, tc, key_d, keyt_d, wk_d, v_d, qpb_d, gum_d, ctx_d, aln_d)
    nc.compile()
    return nc


def make_in_maps(query, key, w_conv, w_q, w_k, v, b, gumbel):
    """Host-side sharding + tiny query-path projection + layout transforms."""
    query = np.asarray(query, np.float32)
    key = np.asarray(key, np.float32)
    w_conv = np.asarray(w_conv, np.float32)
    w_q = np.asarray(w_q, np.float32)
    w_k = np.asarray(w_k, np.float32)
    v = np.asarray(v, np.float32)
    b = np.asarray(b, np.float32)
    gumbel = np.asarray(gumbel, np.float32)

    qp = (query.reshape(N, C) @ w_conv.T) @ w_q + b.reshape(1, U)  # (N, U)
    # gumbel columns: gum[n, p, i] = gumbel[n, i*128 + p] - GSHIFT
    gum_t = np.ascontiguousarray(
        (gumbel.reshape(N, NT, P) - GSHIFT).transpose(0, 2, 1)
    )  # (N, 128, 64)
    key16 = np.ascontiguousarray(key.astype(np.float16))
    keyt16 = np.ascontiguousarray(key16.transpose(0, 2, 1))
    wk16 = np.ascontiguousarray(w_k.astype(np.float16))
    v16 = np.ascontiguousarray(
        np.repeat(v.reshape(U, 1), 2, axis=1).astype(np.float16)
    )

    in_maps = []
    for i in range(NCORES):
        sl = slice(i * NB, (i + 1) * NB)
        in_maps.append(
            {
                "key": key16[sl],
                "keyt": keyt16[sl],
                "wk": wk16,
                "v": v16,
                "qpb": np.ascontiguousarray(qp[sl].T),  # (128, NB)
                "gum": np.ascontiguousarray(gum_t[sl]),
            }
        )
    return in_maps


def gather_outputs(results):
    context = np.empty((N, 1, C), np.float32)
    align = np.empty((N, 1, T), np.float32)
    for i, r in enumerate(results):
        sl = slice(i * NB, (i + 1) * NB)
        context[sl, 0, :] = r["ctx"]
        align[sl, 0, :] = r["aln"].transpose(0, 2, 1).reshape(NB, T)
    return context, align


_NC_CACHE = None


def kernel(query, key, w_conv, w_q, w_k, v, b, gumbel):
    global _NC_CACHE
    if _NC_CACHE is None:
        _NC_CACHE = build_nc()
    in_maps = make_in_maps(query, key, w_conv, w_q, w_k, v, b, gumbel)
    res = run_bass_kernel_spmd(_NC_CACHE, in_maps, core_ids=list(range(NCORES)))
    return gather_outputs(res.results)


# revision 8
# speedup vs baseline: 2.5980x; 1.0378x over previous
"""Bahdanau attention (gumbel-softmax) Trainium2 kernel.

Data-parallel over the batch dim N across 8 NeuronCores (4 batches per core).
Per batch, a single pass over `key` (the only large tensor), fp16 on-chip
compute with fp32 PSUM accumulation:

  per 128-row key tile (t-tile):
    PE:  kp[u,t]   = w_k.T @ keyT                      (contract c, fp32 accum)
    ACT: tanh_ut   = tanh(kp + qp[n] + b)              (fused per-partition bias,
                                                        fp16 out)
    PE:  score[t,2]= tanh_ut.T @ [v v]                 (contract u, fp32 out)
    ACT: e[t]      = exp(score + gumbel[t] - 10)       (shift keeps e in fp16
                                                        range; cancels in softmax)
    PE:  ctx[1,c] += e16.T @ key_tile                  (online softmax numerator)
  batch epilogue: denom = sum(e); ctx/denom, e/denom; DMA out.

The kp matmul contracts over channels (needs key as [c, t]) while the context
matmul contracts over time (needs key as [t, c]), so the host ships key in
both layouts, cast to fp16 — 2 x 16.8 MiB per core, the same byte volume as
the single fp32 copy, halving effective HBM pressure per layout
(target_regime=memory). All reductions accumulate in fp32; align is produced
from the fp32 exp. The gumbel shift (-10) cancels exactly in the softmax
normalization.

Host side does only the tiny query-path projection (q @ w_conv @ w_q + b,
~6 MFLOP on 32x256 data), dtype/layout preparation, and output gather.
"""

import numpy as np
from contextlib import ExitStack

from concourse import bacc, mybir, tile
from concourse.bass_utils import run_bass_kernel_spmd

F32 = mybir.dt.float32
F16 = mybir.dt.float16
AF = mybir.ActivationFunctionType
AX = mybir.AxisListType

N, T, C, U = 32, 8192, 256, 128
NCORES = 8
NB = N // NCORES            # batches per core
P = 128                     # partition / tile size
NT = T // P                 # 64 t-tiles per batch
GROUP = 4                   # t-tiles per compute group
NG = NT // GROUP            # 16 groups per batch
BLK = 8                     # t-tiles per DMA block (0.5 MiB fp16 per stream)
NBLK = NT // BLK            # 8 DMA blocks per batch

GSHIFT = 10.0               # gumbel shift (applied on host) for fp16 exp range


def _emit_kernel(ctx: ExitStack, tc, key_d, keyt_d, wk_d, v_d, qpb_d, gum_d,
                 ctx_d, aln_d):
    nc = tc.nc

    const = ctx.enter_context(tc.tile_pool(name="const", bufs=1))
    wk_sb = const.tile([P, 2, U], F16)          # [c-in-chunk, c-chunk, u]
    nc.sync.dma_start(wk_sb[:, 0, :], wk_d[0:P, :])
    nc.sync.dma_start(wk_sb[:, 1, :], wk_d[P : 2 * P, :])
    v2_sb = const.tile([U, 2], F16)
    nc.sync.dma_start(v2_sb[:], v_d[:])
    qpb_sb = const.tile([U, NB], F32)
    nc.sync.dma_start(qpb_sb[:], qpb_d[:])
    ones_col = const.tile([P, 1], F32)
    nc.gpsimd.memset(ones_col[:], 1.0)
    ones_row = const.tile([1, P], F32)
    nc.gpsimd.memset(ones_row[:], 1.0)

    gum_pool = ctx.enter_context(tc.tile_pool(name="gum", bufs=2))
    key_pool = ctx.enter_context(tc.tile_pool(name="key", bufs=3))
    keyt_pool = ctx.enter_context(tc.tile_pool(name="keyt", bufs=2))
    tanh_pool = ctx.enter_context(tc.tile_pool(name="tanh", bufs=3))
    e_pool = ctx.enter_context(tc.tile_pool(name="e", bufs=2))
    z_pool = ctx.enter_context(tc.tile_pool(name="z", bufs=2))
    aout_pool = ctx.enter_context(tc.tile_pool(name="aout", bufs=2))
    cout_pool = ctx.enter_context(tc.tile_pool(name="cout", bufs=2))
    mini_pool = ctx.enter_context(tc.tile_pool(name="mini", bufs=2))

    kp_pool = ctx.enter_context(tc.tile_pool(name="kp", bufs=3, space="PSUM"))
    sc_pool = ctx.enter_context(tc.tile_pool(name="sc", bufs=2, space="PSUM"))
    cx_pool = ctx.enter_context(tc.tile_pool(name="cx", bufs=1, space="PSUM"))
    ep_pool = ctx.enter_context(tc.tile_pool(name="ep", bufs=1, space="PSUM"))

    for n in range(NB):
        gum_sb = gum_pool.tile([P, NT], F32)
        nc.sync.dma_start(gum_sb[:], gum_d[n])
        e_sb = e_pool.tile([P, NT], F32, tag="e32")
        e16_sb = e_pool.tile([P, NT], F16, tag="e16")
        cx_ps = cx_pool.tile([1, C], F32)

        key_tiles = {}
        keyt_tiles = {}
        tanh_tiles = {}

        def load_block(b):
            key_sb = key_pool.tile([P, BLK, C], F16)
            src = key_d[n, b * BLK * P : (b + 1) * BLK * P, :].rearrange(
                "(p j) c -> p j c", p=P
            )
            nc.sync.dma_start(key_sb[:], src)
            key_tiles[b] = key_sb
            if b % 2 == 0:
                keyt_sb = keyt_pool.tile([P, 2, 2 * BLK * P], F16)
                srct = keyt_d[n].rearrange("(cc p) t -> p cc t", p=P)[
                    :, :, b * BLK * P : (b + 2) * BLK * P
                ]
                nc.sync.dma_start(keyt_sb[:], srct)
                keyt_tiles[b] = keyt_sb
                keyt_tiles[b + 1] = None  # second half of the same tile

        def front(g):
            # kp matmul + tanh for group g
            b, h = divmod(g, BLK // GROUP)
            keyt_sb = keyt_tiles[b - b % 2]
            hh = (b % 2) * (BLK // GROUP) + h
            kp_ps = kp_pool.tile([U, GROUP * P], F32)
            for cc in range(2):
                nc.tensor.matmul(
                    kp_ps[:],
                    wk_sb[:, cc, :],
                    keyt_sb[:, cc, hh * GROUP * P : (hh + 1) * GROUP * P],
                    start=(cc == 0),
                    stop=(cc == 1),
                )
            th_sb = tanh_pool.tile([U, GROUP * P], F16)
            nc.scalar.activation(
                th_sb[:], kp_ps[:], AF.Tanh, bias=qpb_sb[:, n : n + 1]
            )
            tanh_tiles[g] = th_sb

        def score(g):
            th_sb = tanh_tiles.pop(g)
            sc_ps = sc_pool.tile([P, GROUP, 2], F32)
            for j in range(GROUP):
                nc.tensor.matmul(
                    sc_ps[:, j, :],
                    th_sb[:, j * P : (j + 1) * P],
                    v2_sb[:],
                    start=(j == 0),
                    stop=(j == GROUP - 1),
                )
            z_sb = z_pool.tile([P, GROUP], F32)
            nc.vector.tensor_add(
                z_sb[:], sc_ps[:, :, 0], gum_sb[:, g * GROUP : (g + 1) * GROUP]
            )
            nc.scalar.activation(
                e_sb[:, g * GROUP : (g + 1) * GROUP], z_sb[:], AF.Exp
            )
            nc.vector.tensor_copy(
                e16_sb[:, g * GROUP : (g + 1) * GROUP],
                e_sb[:, g * GROUP : (g + 1) * GROUP],
            )

        def ctx_acc(g):
            b, h = divmod(g, BLK // GROUP)
            key_sb = key_tiles[b]
            for j in range(GROUP):
                i = g * GROUP + j
                nc.tensor.matmul(
                    cx_ps[:],
                    e16_sb[:, i : i + 1],
                    key_sb[:, h * GROUP + j, :],
                    start=(i == 0),
                    stop=(i == NT - 1),
                )

        # software-pipelined emission: PE order per iter g is
        #   kp(g) | cx(g-2) | sc(g-1)
        # so each cross-engine round-trip has a group of PE work to hide in.
        load_block(0)
        for g in range(NG + 2):
            if g < NG:
                if g % (BLK // GROUP) == 0:
                    b_next = g // (BLK // GROUP) + 1
                    if b_next < NBLK:
                        load_block(b_next)
                front(g)
            if g >= 2:
                ctx_acc(g - 2)
            if 1 <= g <= NG:
                score(g - 1)

        # batch epilogue: denominator + normalization (tiny fp32 matmuls)
        den_ps = ep_pool.tile([1, NT], F32, tag="ep")
        nc.tensor.matmul(den_ps[:], ones_col[:], e_sb[:], start=True, stop=True)
        s_sb = mini_pool.tile([1, 1], F32, tag="ssum")
        nc.vector.reduce_sum(s_sb[:], den_ps[:], axis=AX.X)
        r32_sb = mini_pool.tile([1, 1], F32, tag="recip32")
        nc.vector.reciprocal(r32_sb[:], s_sb[:])
        rb_ps = ep_pool.tile([P, 1], F32, tag="ep")
        nc.tensor.matmul(rb_ps[:], ones_row[:], r32_sb[:], start=True, stop=True)
        rb_sb = mini_pool.tile([P, 1], F32, tag="rb")
        nc.scalar.copy(rb_sb[:], rb_ps[:])

        aln_sb = aout_pool.tile([P, NT], F32)
        nc.vector.tensor_scalar_mul(aln_sb[:], e_sb[:], rb_sb[:])
        nc.sync.dma_start(aln_d[n], aln_sb[:])
        cxo_sb = cout_pool.tile([1, C], F32)
        nc.vector.tensor_scalar_mul(cxo_sb[:], cx_ps[:], r32_sb[:])
        nc.sync.dma_start(ctx_d[n : n + 1, :], cxo_sb[:])


def build_nc():
    nc = bacc.Bacc("TRN2", target_bir_lowering=False)
    key_d = nc.dram_tensor("key", [NB, T, C], F16, kind="ExternalInput")
    keyt_d = nc.dram_tensor("keyt", [NB, C, T], F16, kind="ExternalInput")
    wk_d = nc.dram_tensor("wk", [C, U], F16, kind="ExternalInput")
    v_d = nc.dram_tensor("v", [U, 2], F16, kind="ExternalInput")
    qpb_d = nc.dram_tensor("qpb", [U, NB], F32, kind="ExternalInput")
    gum_d = nc.dram_tensor("gum", [NB, P, NT], F32, kind="ExternalInput")
    ctx_d = nc.dram_tensor("ctx", [NB, C], F32, kind="ExternalOutput")
    aln_d = nc.dram_tensor("aln", [NB, P, NT], F32, kind="ExternalOutput")
    with tile.TileContext(nc) as tc, ExitStack() as ctx:
        _emit_kernel(ctx, tc, key_d, keyt_d, wk_d, v_d, qpb_d, gum_d, ctx_d, aln_d)
    nc.compile()
    return nc


def make_in_maps(query, key, w_conv, w_q, w_k, v, b, gumbel):
    """Host-side sharding + tiny query-path projection + layout transforms."""
    query = np.asarray(query, np.float32)
    key = np.asarray(key, np.float32)
    w_conv = np.asarray(w_conv, np.float32)
    w_q = np.asarray(w_q, np.float32)
    w_k = np.asarray(w_k, np.float32)
    v = np.asarray(v, np.float32)
    b = np.asarray(b, np.float32)
    gumbel = np.asarray(gumbel, np.float32)

    qp = (query.reshape(N, C) @ w_conv.T) @ w_q + b.reshape(1, U)  # (N, U)
    # p-major rows: natural-key partition p of block b holds rows
    # t = b*1024 + p*8 + j (4KB bursts); e/gum column i = b*8 + j.
    # keyt column X = b*1024 + h*512 + jj*128 + p must hold the same row's
    # channels, i.e. key[t = b*1024 + p*8 + h*4 + jj, :].
    gum_t = np.ascontiguousarray(
        (gumbel.reshape(N, NBLK, P, BLK) - GSHIFT)
        .transpose(0, 2, 1, 3)
        .reshape(N, P, NT)
    )  # gum[n, p, b*8+j] = gumbel[n, b*1024+p*8+j] - GSHIFT
    key16 = np.ascontiguousarray(key.astype(np.float16))
    perm = (
        np.arange(T)
        .reshape(NBLK, P, 2, 4)  # [b][p][h][jj] = b*1024 + p*8 + h*4 + jj
        .transpose(0, 2, 3, 1)   # [b][h][jj][p]
        .reshape(T)
    )
    keyt16 = np.ascontiguousarray(key16[:, perm, :].transpose(0, 2, 1))
    wk16 = np.ascontiguousarray(w_k.astype(np.float16))
    v16 = np.ascontiguousarray(
        np.repeat(v.reshape(U, 1), 2, axis=1).astype(np.float16)
    )

    in_maps = []
    for i in range(NCORES):
        sl = slice(i * NB, (i + 1) * NB)
        in_maps.append(
            {
                "key": key16[sl],
                "keyt": keyt16[sl],
                "wk": wk16,
                "v": v16,
                "qpb": np.ascontiguousarray(qp[sl].T),  # (128, NB)
                "gum": np.ascontiguousarray(gum_t[sl]),
            }
        )
    return in_maps


def gather_outputs(results):
    context = np.empty((N, 1, C), np.float32)
    align = np.empty((N, 1, T), np.float32)
    for i, r in enumerate(results):
        sl = slice(i * NB, (i + 1) * NB)
        context[sl, 0, :] = r["ctx"]
        # aln[n, p, b*8+j] = align[n, b*1024 + p*8 + j]
        a = r["aln"].reshape(NB, P, NBLK, BLK).transpose(0, 2, 1, 3)
        align[sl, 0, :] = a.reshape(NB, T)
    return context, align


_NC_CACHE = None


def kernel(query, key, w_conv, w_q, w_k, v, b, gumbel):
    global _NC_CACHE
    if _NC_CACHE is None:
        _NC_CACHE = build_nc()
    in_maps = make_in_maps(query, key, w_conv, w_q, w_k, v, b, gumbel)
    res = run_bass_kernel_spmd(_NC_CACHE, in_maps, core_ids=list(range(NCORES)))
    return gather_outputs(res.results)
